# revision 21
# baseline (speedup 1.0000x reference)
"""Causal multi-head attention (B=4, T=2048, C=1024, H=16) on 8 TRN2 cores.

Sharding: batch (4) x head-group (2 groups of 8 heads) -> 8 shards, one per
core. Each core computes QKV projections for its 8 heads, causal attention,
and a Megatron row-parallel slice of the output projection; the host sums the
two head-group partial outputs per batch element.

Design (evolved from the f32r baseline, ~345us/iter -> ~105-150us/iter):
  - all matmul operands bf16 (f32 PSUM accumulation); rel err ~5.6e-3
  - K kept resident in SBUF (no DRAM spill round-trip)
  - transposed PV: ctx[q, d] = P^T.T @ [V | ones], so the softmax
    denominator arrives as psum column 64 and normalization is a
    per-partition reciprocal+scale on DVE (no broadcast matmuls)
  - ctx -> ctx^T via the DMA xbar transpose unit (sbuf->sbuf), freeing
    PE and DVE from the transpose chain entirely
  - trimmed causal staircase (widths 512/384/256/128) with a packed
    bf16 mask applied on DVE in 4x perf mode
  - emission interleaves the two heads of each pair unit-by-unit, keeps
    a one-superblock lag between S/exp and PV, and paces V/QK/output
    projection groups as PE filler between exp-gated tiles; the output
    projection streams per query superblock during the last head pair

Self-contained: hardcodes shapes from the problem spec; no file reads.
"""
import sys
sys.path.insert(0, '/opt/trn_rl_repo')
import numpy as np
import ml_dtypes

BF = ml_dtypes.bfloat16

B, T, C = 4, 2048, 1024
H, D = 16, 64
N_CORES = 8
HPC = 8        # heads per core
HP = 4         # head pairs per core
KB = 16        # 128-row key tiles per sequence
NQSB = 4       # 512-column query superblocks
CI = 8         # 128-row contraction tiles over C
VW = 65        # V_aug stride per head (64 V cols + 1 ones col)

# Diagonal staircase inside one query superblock: key tile j (local) covers
# queries [QOFF[j], 512). Packed psum/mask layout: j0,j1 in tile A at cols
# 0/512; j2,j3 in tile B at cols 0/256. Mask columns: A then B.
QOFF = (0, 128, 256, 384)
DW = (512, 384, 256, 128)
AOFF = (0, 512)        # diag A packing (j0, j1) -> 896 cols
BOFF = (0, 256)        # diag B packing (j2, j3) -> 384 cols
MW = 1280              # mask width: 896 (A) + 384 (B)

_CACHE = {}


def build_nc(iters=1, psum_split=(3, 2), interleave=True,
             rates=(0.55, 0.15, 0.7), qk_on_act=True):
    import contextlib
    from collections import deque
    import concourse.tile as tile
    from concourse import bacc, mybir

    F32 = mybir.dt.float32
    BF16 = mybir.dt.bfloat16
    EXP = mybir.ActivationFunctionType.Exp

    nc = bacc.Bacc("TRN2", target_bir_lowering=False, debug=False)

    xT_d = nc.dram_tensor("xT", [C, T], BF16, kind="ExternalInput")
    wqT_d = nc.dram_tensor("wqT", [C, 512], BF16, kind="ExternalInput")
    wkT_d = nc.dram_tensor("wkT", [C, 512], BF16, kind="ExternalInput")
    wvT_d = nc.dram_tensor("wvT", [C, 512], BF16, kind="ExternalInput")
    woT_d = nc.dram_tensor("woT", [512, C], BF16, kind="ExternalInput")
    bias_d = nc.dram_tensor("bias", [128, 8], F32, kind="ExternalInput")
    mask_d = nc.dram_tensor("masks", [128, MW], BF16, kind="ExternalInput")
    yT_d = nc.dram_tensor("yT", [C, T], F32, kind="ExternalOutput")

    with tile.TileContext(nc) as tc:
        def emit():
            with contextlib.ExitStack() as es:
                const = es.enter_context(tc.tile_pool(name="const", bufs=1))
                bigp = es.enter_context(tc.tile_pool(name="big", bufs=1))
                ptp = es.enter_context(tc.tile_pool(name="ptp", bufs=24))
                stagep = es.enter_context(tc.tile_pool(name="stg", bufs=1))
                recipp = es.enter_context(tc.tile_pool(name="rcp", bufs=4))
                yp = es.enter_context(tc.tile_pool(name="y", bufs=3))
                sps = es.enter_context(
                    tc.tile_pool(name="sps", bufs=psum_split[0],
                                 space="PSUM"))
                cps = es.enter_context(
                    tc.tile_pool(name="cps", bufs=psum_split[1],
                                 space="PSUM"))

                bias_sb = const.tile([128, 8], F32)
                mask_sb = const.tile([128, MW], BF16)

                xt_sb, wv_sb, wq_sb, wk_sb = [], [], [], []
                for ci in range(CI):
                    t_ = bigp.tile([128, T], BF16, tag=f"xt{ci}")
                    nc.sync.dma_start(t_[:],
                                      xT_d.ap()[ci * 128:(ci + 1) * 128, :])
                    xt_sb.append(t_)
                for ci in range(CI):
                    t_ = bigp.tile([128, 512], BF16, tag=f"wv{ci}")
                    nc.sync.dma_start(t_[:],
                                      wvT_d.ap()[ci * 128:(ci + 1) * 128, :])
                    wv_sb.append(t_)
                for ci in range(CI):
                    t_ = bigp.tile([128, 512], BF16, tag=f"wq{ci}")
                    nc.sync.dma_start(t_[:],
                                      wqT_d.ap()[ci * 128:(ci + 1) * 128, :])
                    wq_sb.append(t_)
                for ci in range(CI):
                    t_ = bigp.tile([128, 512], BF16, tag=f"wk{ci}")
                    nc.sync.dma_start(t_[:],
                                      wkT_d.ap()[ci * 128:(ci + 1) * 128, :])
                    wk_sb.append(t_)
                nc.sync.dma_start(bias_sb[:], bias_d.ap())
                nc.sync.dma_start(mask_sb[:], mask_d.ap())
                wo_sb = []
                for hp in range(HP):
                    t_ = bigp.tile([128, C], BF16, tag=f"wo{hp}")
                    nc.sync.dma_start(t_[:],
                                      woT_d.ap()[hp * 128:(hp + 1) * 128, :])
                    wo_sb.append(t_)

                qt_sb, kt_sb, ctx_sb, v_sb = [], [], [], []
                for hp in range(HP):
                    qt_sb.append(bigp.tile([128, T], BF16, tag=f"qt{hp}",
                                           name=f"qt{hp}"))
                    kt_sb.append(bigp.tile([128, T], BF16, tag=f"kt{hp}",
                                           name=f"kt{hp}"))
                    ctx_sb.append(bigp.tile([128, T], BF16, tag=f"ctx{hp}",
                                            name=f"ctx{hp}"))
                for kb in range(KB):
                    v_sb.append(bigp.tile([128, HPC * VW], BF16,
                                          tag=f"v{kb}", name=f"v{kb}"))
                stage_sb = [stagep.tile([128, 128], BF16, tag=f"st{q}",
                                        name=f"st{q}")
                            for q in range(KB)]

                # ---------------- PE work-group emitters ----------------
                def v_group(ti):
                    ps_ = sps.tile([128, 1024], F32, tag="sp", name="sp")
                    for ci in range(CI):
                        nc.tensor.matmul(
                            ps_[:, 0:512],
                            xt_sb[ci][:, ti * 128:(ti + 1) * 128],
                            wv_sb[ci][:],
                            start=(ci == 0), stop=(ci == CI - 1),
                            skip_group_check=True)
                    sv = v_sb[ti][:].rearrange("p (h w) -> p h w", w=VW)
                    pv = ps_[:, 0:512].rearrange("p (h w) -> p h w", w=64)
                    nc.vector.tensor_copy(sv[:, :, 0:64], pv)
                    nc.gpsimd.memset(sv[:, :, 64:65], 1.0)

                def qk_group(hp, tj, which):
                    w_sb = wq_sb if which == 'q' else wk_sb
                    dst = qt_sb[hp] if which == 'q' else kt_sb[hp]
                    fsl = slice(hp * 128, (hp + 1) * 128)
                    tsl = slice(tj * 512, (tj + 1) * 512)
                    ps_ = sps.tile([128, 1024], F32, tag="sp", name="sp")
                    for ci in range(CI):
                        nc.tensor.matmul(
                            ps_[:, 0:512], w_sb[ci][:, fsl],
                            xt_sb[ci][:, tsl],
                            start=(ci == 0), stop=(ci == CI - 1),
                            skip_group_check=True)
                    if qk_on_act:
                        nc.scalar.copy(dst[:, tsl], ps_[:, 0:512])
                    else:
                        nc.vector.tensor_copy(dst[:, tsl], ps_[:, 0:512])

                def p3_group(tj, oi):
                    osl = slice(oi * 128, (oi + 1) * 128)
                    tsl = slice(tj * 512, (tj + 1) * 512)
                    ps_ = sps.tile([128, 1024], F32, tag="sp", name="sp")
                    for hp in range(HP):
                        nc.tensor.matmul(
                            ps_[:, 0:512], wo_sb[hp][:, osl],
                            ctx_sb[hp][:, tsl],
                            start=(hp == 0), stop=(hp == HP - 1),
                            skip_group_check=True)
                    y_ = yp.tile([128, 512], F32, name="yt")
                    nc.vector.tensor_scalar_add(y_[:], ps_[:, 0:512],
                                                bias_sb[:, oi:oi + 1])
                    nc.sync.dma_start(yT_d.ap()[osl, tsl], y_[:])

                # ---------------- interleave machinery ----------------
                filler = deque()
                state = {"acc": 0.0, "rate": 0.0}

                def point():
                    state["acc"] += state["rate"]
                    while state["acc"] >= 1.0 and filler:
                        state["acc"] -= 1.0
                        filler.popleft()()

                # ---------------- prologue ----------------
                # V for key tiles 0..7 and Q/K for head pair 0 up front;
                # the rest of V and later head pairs' Q/K interleave with
                # attention as PE filler between exp-gated S tiles.
                for ti in range(8):
                    v_group(ti)
                for tj in range(NQSB):
                    qk_group(0, tj, 'q')
                    qk_group(0, tj, 'k')

                # ---------------- attention ----------------
                for hp in range(HP):
                    if hp == 0:
                        for ti in range(8, KB):
                            filler.append(lambda ti=ti: v_group(ti))
                        for tj in range(NQSB):
                            filler.append(
                                lambda tj=tj: qk_group(1, tj, 'q'))
                            filler.append(
                                lambda tj=tj: qk_group(1, tj, 'k'))
                        state["rate"] = rates[0]
                    elif hp < HP - 1:
                        for tj in range(NQSB):
                            filler.append(
                                lambda tj=tj, hp=hp: qk_group(hp + 1, tj, 'q'))
                            filler.append(
                                lambda tj=tj, hp=hp: qk_group(hp + 1, tj, 'k'))
                        state["rate"] = rates[1]
                    else:
                        state["rate"] = rates[2]

                    def s_pair(hp, hl, qsb, p2, ptloc):
                        prow = slice(hl * 64, hl * 64 + 64)
                        qbase = qsb * 512
                        ps_ = sps.tile([128, 1024], F32, tag="sp", name="sp")
                        for u in range(2):
                            kb = 2 * p2 + u
                            nc.tensor.matmul(
                                ps_[:, u * 512:(u + 1) * 512],
                                kt_sb[hp][prow, kb * 128:(kb + 1) * 128],
                                qt_sb[hp][prow, qbase:qbase + 512],
                                start=True, stop=True,
                                skip_group_check=True)
                        pt = ptp.tile([128, 1024], BF16, tag="pt", name="pt")
                        nc.scalar.activation(pt[:], ps_[:], EXP, scale=0.125)
                        for u in range(2):
                            ptloc[2 * p2 + u] = (pt, u * 512, 0)

                    def s_diag(hp, hl, qsb, half, ptloc):
                        prow = slice(hl * 64, hl * 64 + 64)
                        qbase = qsb * 512
                        nf = 4 * qsb
                        js = (0, 1) if half == 0 else (2, 3)
                        offs = AOFF if half == 0 else BOFF
                        w = 896 if half == 0 else 384
                        moff = 0 if half == 0 else 896
                        ps_ = sps.tile([128, 1024], F32, tag="sp", name="sp")
                        for u, j in enumerate(js):
                            kb = nf + j
                            nc.tensor.matmul(
                                ps_[:, offs[u]:offs[u] + DW[j]],
                                kt_sb[hp][prow, kb * 128:(kb + 1) * 128],
                                qt_sb[hp][prow,
                                          qbase + QOFF[j]:qbase + 512],
                                start=True, stop=True,
                                skip_group_check=True)
                        pt = ptp.tile([128, 1024], BF16, tag="pt", name="pt")
                        nc.scalar.activation(pt[:, 0:w], ps_[:, 0:w],
                                             EXP, scale=0.125)
                        nc.vector.tensor_mul(pt[:, 0:w], pt[:, 0:w],
                                             mask_sb[:, moff:moff + w])
                        for u, j in enumerate(js):
                            ptloc[nf + j] = (pt, offs[u], QOFF[j])

                    def pv_qt(hp, hl, qsb, qt, ptloc):
                        h = 2 * hp + hl
                        vsl = slice(h * VW, h * VW + VW)
                        qi = 4 * qsb + qt
                        qcol = qt * 128
                        ct = cps.tile([128, 512], F32, tag="cp", name="cp")
                        for kb in range(qi + 1):
                            pt, base, qs = ptloc[kb]
                            c0 = base + qcol - qs
                            nc.tensor.matmul(
                                ct[:, 0:VW],
                                pt[:, c0:c0 + 128],
                                v_sb[kb][:, vsl],
                                start=(kb == 0), stop=(kb == qi),
                                skip_group_check=True)
                        rc = recipp.tile([128, 1], F32, name="rc")
                        nc.vector.reciprocal(rc[:], ct[:, 64:65])
                        st = stage_sb[qsb * 4 + qt]
                        nc.vector.tensor_scalar_mul(
                            st[:, hl * 64:hl * 64 + 64], ct[:, 0:64], rc[:])
                        if hl == 1:
                            # ctx^T via the DMA xbar transpose unit: frees
                            # PE (no identity matmul) and DVE (no psum copy)
                            nc.sync.dma_start_transpose(
                                ctx_sb[hp][:, qsb * 512 + qcol:
                                           qsb * 512 + qcol + 128],
                                st[:])

                    def head_units(hp, hl):
                        """Per-head emission units (closures), with the
                        one-superblock PV lag built in. Marker = qsb index
                        on the last PV chunk of that superblock."""
                        units = []
                        pend = []
                        for qsb in range(NQSB):
                            ptloc = {}
                            for p2 in range(2 * qsb):
                                units.append((lambda hp=hp, hl=hl, qsb=qsb,
                                              p2=p2, pl=ptloc:
                                              s_pair(hp, hl, qsb, p2, pl),
                                              None))
                                if pend:
                                    units.append(pend.pop(0))
                            for half in range(2):
                                units.append((lambda hp=hp, hl=hl, qsb=qsb,
                                              half=half, pl=ptloc:
                                              s_diag(hp, hl, qsb, half, pl),
                                              None))
                                if pend:
                                    units.append(pend.pop(0))
                            while pend:
                                units.append(pend.pop(0))
                            pend = [(lambda hp=hp, hl=hl, qsb=qsb, qt=qt,
                                     pl=ptloc: pv_qt(hp, hl, qsb, qt, pl),
                                     qsb if qt == NQSB - 1 else None)
                                    for qt in range(NQSB)]
                        while pend:
                            units.append(pend.pop(0))
                        return units

                    # interleave the pair's two heads unit-by-unit (h0 first
                    # at each step: h1's transpose reads h0's stage writes)
                    u0 = head_units(hp, 0)
                    u1 = head_units(hp, 1)
                    if interleave:
                        merged = [(units, i)
                                  for i in range(max(len(u0), len(u1)))
                                  for units in (u0, u1) if i < len(units)]
                    else:
                        merged = ([(u0, i) for i in range(len(u0))]
                                  + [(u1, i) for i in range(len(u1))])
                    for units, i in merged:
                        if True:
                            cl, marker = units[i]
                            cl()
                            point()
                            if (units is u1 and marker is not None
                                    and hp == HP - 1):
                                # ctx^T for this query superblock complete
                                # across all head pairs: emit its output
                                # projection as filler
                                for oi in range(8):
                                    filler.append(
                                        lambda oi=oi, tj=marker:
                                        p3_group(tj, oi))

                # drain whatever output-projection filler remains
                while filler:
                    filler.popleft()()

        if iters == 1:
            emit()
        else:
            with tc.For_i(0, iters, 1):
                emit()
    nc.compile()
    return nc


def make_masks():
    """Merged staircase mask [128, MW] (bf16 0/1): diag block j covers
    queries [QOFF[j], 512); keep iff q >= 128*j + k."""
    m = np.zeros((128, MW), np.float32)
    moffs = (0, 512, 896, 1152)
    k = np.arange(128)[:, None]
    for j in range(4):
        q = np.arange(QOFF[j], 512)[None, :]
        m[:, moffs[j]:moffs[j] + DW[j]] = (q >= 128 * j + k)
    return m.astype(BF)


def shard_inputs(x, w_qkv, w_out, b_out):
    """Full inputs -> list of 8 per-core input dicts (bf16 operands)."""
    x = np.asarray(x, dtype=np.float32)
    w_qkv = np.asarray(w_qkv, dtype=np.float32)
    w_out = np.asarray(w_out, dtype=np.float32)
    b_out = np.asarray(b_out, dtype=np.float32)
    masks = make_masks()
    in_maps = []
    for c in range(N_CORES):
        b, hg = c // 2, c % 2
        h0 = hg * HPC
        csl = slice(h0 * D, (h0 + HPC) * D)
        im = {
            "xT": np.ascontiguousarray(x[b].T).astype(BF),
            "wqT": np.ascontiguousarray(w_qkv[0 * C:1 * C][csl].T).astype(BF),
            "wkT": np.ascontiguousarray(w_qkv[1 * C:2 * C][csl].T).astype(BF),
            "wvT": np.ascontiguousarray(w_qkv[2 * C:3 * C][csl].T).astype(BF),
            "woT": np.ascontiguousarray(w_out[:, csl].T).astype(BF),
            "bias": (np.ascontiguousarray(b_out.reshape(8, 128).T)
                     if hg == 0 else np.zeros((128, 8), np.float32)),
            "masks": masks,
        }
        in_maps.append(im)
    return in_maps


def gather_outputs(results):
    """8 per-core {'yT': [C,T]} -> full [B,T,C]."""
    y = np.empty((B, T, C), np.float32)
    for b in range(B):
        acc = results[2 * b]["yT"] + results[2 * b + 1]["yT"]
        y[b] = acc.T
    return y


def kernel(**inputs):
    from concourse.bass_utils import run_bass_kernel_spmd
    if "nc" not in _CACHE:
        _CACHE["nc"] = build_nc()
    nc = _CACHE["nc"]
    in_maps = shard_inputs(inputs["x"], inputs["w_qkv"],
                           inputs["w_out"], inputs["b_out"])
    res = run_bass_kernel_spmd(nc, in_maps, list(range(N_CORES)))
    return gather_outputs(res.results)
